# revision 21
# baseline (speedup 1.0000x reference)
"""Trainium2 Bass kernel for MoE-with-LoRA-experts (nn_MoE_64098091925598).

Reference computation (N=8192 tokens, D=1024, E=8 experts, R=16, top-2):
    logits  = x @ W_gate.T                      [N, E]
    combine = scatter(softmax(top2(logits)))    [N, E] (2 nonzeros/row)
    moe     = sum_e combine[:,e] * (x @ A_e @ B_e)
    out     = moe + x @ W_base.T + b_base

Strategy: data-parallel over tokens across 8 NeuronCores (1024 tokens per
core); every core computes all 8 LoRA experts densely and masks by the
combine weights (the expert FLOPs are tiny vs the base linear, so
expert-parallel all-to-all would be pure overhead).

Optimizations vs the fp32r baseline (76us -> ~67-70us measured):
- Mixed precision: gating keeps exact fp32 x (f32r matmuls stream 1
  cyc/row at free-dim 512, same as bf16, and bf16 logits flip the top-2
  selection vs the fp32 reference on ~1% of tokens, blowing the max-err
  gate). Everything else (W_base, B, H, output) is bf16, halving those
  DMA bytes. The main-loop stationary x operand is a zero-copy strided
  bitcast view of the fp32 tiles (bf16 = high 2 bytes of fp32).
- DMA layout: every load is a [128, contiguous-run] tile = 128 large
  descriptors. Small-row loads (wg 256B rows) complete ~7us late
  (descriptors concentrate on few DMA engines), so wg and a are folded
  INTO the x wave-0 rows host-side and W_base is host-packed per
  partition. Few dma_starts total (the ~19-deep DMA-semaphore pool
  otherwise serializes new issues behind unrelated transfers).
- Loads are need-ordered: x wave-0 (sync ring carries chunks {0,2},{1,3}
  which the gating consumes first; the scalar ring starts ~2us later and
  carries {4,6},{5,7}), then W_base half-0, x wave-1, W_base half-1.
- PE program order keeps the tensor engine dense: warm matmuls cover the
  preamble; main-loop base matmuls interleave under the gating/combine
  DVE chain latency; any HAM clock-gate re-throttle (triggered by DMA
  waits) lands in the DMA-paced phase and expires before the PE-bound
  main loop.
- One sigmoid per wave: combine = eq2 + w1*(eq1-eq2) with w1+w2=1.
- Output stored as bf16 (host converts back to fp32); the last tile's
  bias-add + store is split in halves across both rings to shorten the
  drain tail.

Key algebraic trick (unchanged): with H[n,(e,r)] = combine[n,e]*(x@A_e)[n,r]
stacked over experts, the weighted expert sum collapses to one dense K=128
matmul moe = H @ B_flat accumulated into the same PSUM as the base linear.
"""

import numpy as np

import concourse.mybir as mybir
import concourse.tile as tile
from concourse import bacc
from concourse.bass_utils import run_bass_kernel_spmd
from concourse.masks import make_identity

N_TOK, D, E, R, TOPK = 8192, 1024, 8, 16, 2
CORES = 8
NS = N_TOK // CORES  # tokens per core
ER = E * R  # 128, stacked expert-rank dim
DC = D // 128  # 8 contraction chunks
NJ = NS // 128  # 8 token chunks per core
NT = 2  # two 512-token waves
JT = NJ // NT  # 4 token chunks per wave
DT = D // 512  # 2 dout tiles

f32 = mybir.dt.float32
f32r = mybir.dt.float32r
bf16 = mybir.dt.bfloat16
NP_BF16 = mybir.dt.np(bf16)

N_WARM = 20  # bridge preamble -> x0 chunks {1,3} (~14us): sized so the
             # residual front-end DMA-wait gaps stay under the ~3.4us HAM
             # idle window on median runs, avoiding the 1.2GHz re-throttle
             # entirely; on slow-DMA runs the gap absorbs the extra warms
             # at zero cost

_CACHE: dict = {}


def _kernel_body(nc, tc, dram):
    xparts, b_fl, exp_m, b_vec, out = dram

    from contextlib import ExitStack

    ctx = ExitStack()
    pw = ctx.enter_context(tc.tile_pool(name="weights", bufs=1))
    pg = ctx.enter_context(tc.tile_pool(name="gating", bufs=1))
    pmt = ctx.enter_context(tc.tile_pool(name="mmtmp", bufs=2))
    pout = ctx.enter_context(tc.tile_pool(name="outsb", bufs=4))
    ps_tp = ctx.enter_context(tc.tile_pool(name="ps_tp", bufs=2, space="PSUM"))
    ps_mm = ctx.enter_context(tc.tile_pool(name="ps_mm", bufs=2, space="PSUM"))
    ps_out = ctx.enter_context(tc.tile_pool(name="ps_out", bufs=4, space="PSUM"))

    # warm tile init on gpsimd (free early; vector/scalar are busy later)
    warm_sb = pw.tile([128, 512], bf16, tag="warm")
    nc.gpsimd.memset(warm_sb, 0.0)

    # ---- Load phase ------------------------------------------------
    # Every load is a [128, contiguous] tile = 128 big DMA descriptors.
    # Small-descriptor loads (wg 256B rows, per-chunk wb 1KB rows) are
    # pathological: descriptors concentrate on a few DMA engines and the
    # completion semaphore lands ~7us after issue. So wg and a are folded
    # INTO the x wave-0 group rows host-side, and wb is host-packed to
    # 4KB-per-partition runs.
    AUG = 512 + E + ER  # x chunk | wg chunk | a chunk per partition row
    xt = [[None] * NT for _ in range(DC)]
    wgs = [None] * DC
    asb = [None] * DC
    wb = [[None] * DT for _ in range(DC)]

    exp_sb = pw.tile([E, ER], bf16, tag="expand")
    nc.sync.dma_start(out=exp_sb, in_=exp_m)
    b_sb = pw.tile([ER, D], bf16, tag="bflat")
    nc.gpsimd.dma_start(out=b_sb, in_=b_fl)

    def load_xg0(par, half, dram_t):
        cs = [par + 4 * half, par + 4 * half + 2]
        eng = nc.sync if half == 0 else nc.scalar
        g = pw.tile([128, 2, AUG], f32r, tag=f"xg0{par}{half}",
                    name=f"xg0{par}{half}")
        eng.dma_start(out=g, in_=dram_t)
        for k, c in enumerate(cs):
            xt[c][0] = g[:, k, 0:512]
            wgs[c] = g[:, k, 512 : 512 + E]
            asb[c] = g[:, k, 512 + E : AUG]

    def load_xg1(par, half, dram_t):
        cs = [par + 4 * half, par + 4 * half + 2]
        eng = nc.sync if half == 0 else nc.scalar
        g = pw.tile([128, 2, 512], f32r, tag=f"xg1{par}{half}",
                    name=f"xg1{par}{half}")
        eng.dma_start(out=g, in_=dram_t)
        for k, c in enumerate(cs):
            xt[c][1] = g[:, k, :]

    def load_wbg(dt_, par, dram_t):
        # host-packed parity half (contiguous 4KB/partition), one per ring
        eng = nc.sync if par == 0 else nc.scalar
        g = pw.tile([128, 4, 512], bf16, tag=f"wbg{dt_}{par}",
                    name=f"wbg{dt_}{par}")
        eng.dma_start(out=g, in_=dram_t)
        for k in range(4):
            wb[par + 2 * k][dt_] = g[:, k, :]

    (xg000, xg010, xg001, xg011, xg100, xg110, xg101, xg111,
     wb00, wb01, wb10, wb11) = xparts
    load_xg0(0, 0, xg000)   # sync:   chunks {0,2}
    load_xg0(1, 0, xg010)   # sync:   chunks {1,3}
    load_xg0(0, 1, xg001)   # scalar: chunks {4,6}
    load_xg0(1, 1, xg011)   # scalar: chunks {5,7}
    load_wbg(0, 0, wb00)
    load_wbg(0, 1, wb01)
    load_xg1(0, 0, xg100)
    load_xg1(1, 0, xg110)
    load_xg1(0, 1, xg101)
    load_xg1(1, 1, xg111)
    load_wbg(1, 0, wb10)
    load_wbg(1, 1, wb11)

    bias_row = pw.tile([1, D], f32, tag="biasrow")
    nc.gpsimd.dma_start(out=bias_row, in_=b_vec)
    bias_sb = pw.tile([128, D], f32, tag="bias")
    nc.gpsimd.partition_broadcast(bias_sb, bias_row)
    ident = pw.tile([128, 128], f32, tag="ident")
    make_identity(nc, ident)

    # bf16 views of x for the main-loop stationary operand: the high two
    # bytes of each fp32 element ARE its truncated bf16 value, so a
    # strided bitcast view avoids any conversion pass. The strided
    # operand is the stationary one (loaded in the LDWEIGHTS shadow);
    # the moving operand (wb, contiguous bf16) sets the stream rate.
    xb = [[None] * NT for _ in range(DC)]
    for t in range(NT):
        for c in range(DC):
            xb[c][t] = xt[c][t].bitcast(bf16)[:, 1::2]

    # ---- PE prewarm: garbage matmuls, never read --------------------
    warm_ps = ps_tp.tile([128, 512], f32, tag="tp")
    for _ in range(N_WARM):
        nc.tensor.matmul(warm_ps, warm_sb[:, 0:128], warm_sb, start=True, stop=True)

    # per-wave front-end state
    lgT_ps = [None] * NT
    h_ps = [None] * NT
    lgT_sb = [None] * NT
    lg3 = [None] * NT
    mx = [None] * NT
    cb = [None] * NT
    cT_sb = [None] * NT
    h_sb = [None] * NT
    HT = [None] * NT

    def front_mms(t, mid_fill=False):
        """logits^T and h^T = A_flat^T x^T for wave t, chunk-interleaved so
        each x chunk is consumed as it lands."""
        lgT_ps[t] = ps_mm.tile([E, 512], f32, tag="mm", name=f"lgT_ps{t}")
        h_ps[t] = ps_mm.tile([ER, 512], f32, tag="mm", name=f"h_ps{t}")

        def gate(c):
            nc.tensor.matmul(
                lgT_ps[t], wgs[c], xt[c][t],
                start=(c == 0), stop=(c == DC - 1), skip_group_check=True,
            )

        def hmm(c):
            nc.tensor.matmul(
                h_ps[t], asb[c], xt[c][t],
                start=(c == 0), stop=(c == DC - 1), skip_group_check=True,
            )

        for c in (0, 2, 1, 3):
            gate(c)
            hmm(c)
        if mid_fill:
            wfm_ps = ps_tp.tile([128, 512], f32, tag="tp")
            for _ in range(16):
                nc.tensor.matmul(
                    wfm_ps, warm_sb[:, 0:128], warm_sb, start=True, stop=True
                )
        for c in (4, 6, 5, 7):
            gate(c)
            hmm(c)
        h_sb[t] = pmt.tile([ER, 512], f32, tag="hsb", name=f"h_sb{t}")

    def lgT_block(t):
        """token-major logits, top-2, combine weights cb (fp32, exact
        compares); one sigmoid: cb = eq2 + w1*(eq1-eq2)."""
        lgT_sb[t] = pg.tile([E, 512], f32, tag=f"lgT{t}", name=f"lgT_sb{t}")
        nc.vector.tensor_copy(lgT_sb[t], lgT_ps[t])
        lg3[t] = pg.tile([128, JT, E], f32, tag=f"lg3_{t}", name=f"lg3_{t}")
        mx[t] = pg.tile([128, JT, E], f32, tag=f"mx{t}", name=f"mx{t}")
        for r in range(JT):
            tr_ps = ps_tp.tile([128, E], f32, tag="tp")
            nc.tensor.transpose(
                tr_ps, lgT_sb[t][:, r * 128 : (r + 1) * 128], ident[0:E, 0:E]
            )
            nc.vector.tensor_copy(lg3[t][:, r, :], tr_ps)
            nc.vector.max(out=mx[t][:, r, :], in_=lg3[t][:, r, :])
        v1 = mx[t][:, :, 0:1]
        v2 = mx[t][:, :, 1:2]
        d21 = pg.tile([128, JT, 1], f32, tag=f"d21_{t}")
        nc.vector.tensor_sub(d21, v2, v1)
        w1 = pg.tile([128, JT, 1], f32, tag=f"w1_{t}")
        nc.scalar.activation(w1, d21, mybir.ActivationFunctionType.Sigmoid, scale=-1.0)
        nc.scalar.copy(h_sb[t], h_ps[t])  # scalar is free here; frees DVE
        bs = [128, JT, E]
        eq1 = pg.tile(bs, f32, tag=f"eq1_{t}")
        eq2 = pg.tile(bs, f32, tag=f"eq2_{t}")
        nc.vector.tensor_tensor(eq1, lg3[t], v1.to_broadcast(bs), mybir.AluOpType.is_equal)
        nc.vector.tensor_tensor(eq2, lg3[t], v2.to_broadcast(bs), mybir.AluOpType.is_equal)
        cb[t] = pg.tile(bs, f32, tag=f"cb{t}", name=f"cb{t}")
        nc.vector.tensor_sub(eq1, eq1, eq2)
        nc.vector.tensor_tensor(eq1, eq1, w1.to_broadcast(bs), mybir.AluOpType.mult)
        nc.vector.tensor_add(cb[t], eq1, eq2)

    def cT_ce(t):
        """combine^T via PE transpose, expand E->ER via matmul, then
        H^T = expand(combine^T) * h^T (bf16 for the main-loop matmul)."""
        cT_sb[t] = pg.tile([E, 512], bf16, tag=f"cT{t}", name=f"cT_sb{t}")
        for r in range(JT):
            cT_ps = ps_tp.tile([E, 128], f32, tag="tp")
            nc.tensor.transpose(cT_ps, cb[t][:, r, :], ident)
            nc.vector.tensor_copy(cT_sb[t][:, r * 128 : (r + 1) * 128], cT_ps)
        ce_ps = ps_mm.tile([ER, 512], f32, tag="mm")
        nc.tensor.matmul(ce_ps, exp_sb, cT_sb[t], start=True, stop=True)
        HT[t] = pg.tile([ER, 512], bf16, tag=f"HT{t}", name=f"HT{t}")
        nc.vector.tensor_tensor(HT[t], ce_ps, h_sb[t], mybir.AluOpType.mult)

    def base_mms(dt_, j):
        jh, jr = divmod(j, JT)
        ops = ps_out.tile([128, 512], f32, tag="out")
        for c in range(DC):
            nc.tensor.matmul(
                ops,
                xb[c][jh][:, jr * 128 : (jr + 1) * 128],
                wb[c][dt_],
                start=(c == 0),
                stop=False,
            )
        return ops

    def finish(dt_, j, ops):
        dsl = slice(dt_ * 512, (dt_ + 1) * 512)
        jh, jr = divmod(j, JT)
        nc.tensor.matmul(
            ops, HT[jh][:, jr * 128 : (jr + 1) * 128], b_sb[:, dsl],
            start=False, stop=True,
        )
        osb = pout.tile([128, 512], bf16, tag="osb")
        nc.vector.tensor_add(osb, ops, bias_sb[:, dsl])
        eng = nc.sync if (j + dt_) % 2 == 0 else nc.scalar
        eng.dma_start(out=out[j * 128 : (j + 1) * 128, dsl], in_=osb)

    # ---- schedule: keep the PE stream dense end-to-end --------------
    front_mms(0, mid_fill=True)
    lgT_block(0)
    wf_ps = ps_tp.tile([128, 512], f32, tag="tp")
    for _ in range(8):
        nc.tensor.matmul(wf_ps, warm_sb[:, 0:128], warm_sb, start=True, stop=True)
    o0 = base_mms(0, 0)  # covers the wave-0 combine chain latency
    cT_ce(0)
    o1 = base_mms(0, 1)
    finish(0, 0, o0)
    finish(0, 1, o1)
    for j in (2, 3):
        finish(0, j, base_mms(0, j))
    front_mms(1)
    lgT_block(1)
    o4 = base_mms(0, 4)
    cT_ce(1)
    o5 = base_mms(0, 5)
    finish(0, 4, o4)
    finish(0, 5, o5)
    for j in (6, 7):
        finish(0, j, base_mms(0, j))
    for j in range(NJ - 1):
        finish(1, j, base_mms(1, j))
    # last tile: two independent 256-wide half-pipelines so the first
    # half's bias-add + store overlaps the second half's matmuls
    jh, jr = divmod(NJ - 1, JT)
    for hh in range(2):
        dsl_h = slice(512 + hh * 256, 512 + (hh + 1) * 256)
        ops_h = ps_out.tile([128, 256], f32, tag="out", name=f"opsl{hh}")
        for c in range(DC):
            nc.tensor.matmul(
                ops_h,
                xb[c][jh][:, jr * 128 : (jr + 1) * 128],
                wb[c][1][:, hh * 256 : (hh + 1) * 256],
                start=(c == 0),
                stop=False,
            )
        nc.tensor.matmul(
            ops_h, HT[jh][:, jr * 128 : (jr + 1) * 128], b_sb[:, dsl_h],
            start=False, stop=True,
        )
        osb_h = pout.tile([128, 256], bf16, tag=f"osbl{hh}", name=f"osbl{hh}")
        nc.vector.tensor_add(osb_h, ops_h, bias_sb[:, dsl_h])
        eng = nc.sync if hh == 0 else nc.scalar
        eng.dma_start(out=out[(NJ - 1) * 128 : NJ * 128, dsl_h], in_=osb_h)

    ctx.close()


def build_nc():
    nc = bacc.Bacc(
        "TRN2",
        target_bir_lowering=False,
        debug=False,
        enable_asserts=False,
        num_devices=CORES,
    )
    AUG = 512 + E + ER
    xparts = []
    for nm in ("xg000", "xg010", "xg001", "xg011"):
        t = nc.dram_tensor(nm, [128, 2, AUG], f32, kind="ExternalInput").ap()
        xparts.append(t.bitcast(f32r))
    for nm in ("xg100", "xg110", "xg101", "xg111"):
        t = nc.dram_tensor(nm, [128, 2, 512], f32, kind="ExternalInput").ap()
        xparts.append(t.bitcast(f32r))
    for nm in ("wb00", "wb01", "wb10", "wb11"):
        xparts.append(
            nc.dram_tensor(nm, [128, 4, 512], bf16, kind="ExternalInput").ap()
        )
    b_fl = nc.dram_tensor("b_fl", [ER, D], bf16, kind="ExternalInput").ap()
    exp_m = nc.dram_tensor("exp_m", [E, ER], bf16, kind="ExternalInput").ap()
    b_vec = nc.dram_tensor("b_vec", [1, D], f32, kind="ExternalInput").ap()
    out = nc.dram_tensor("out", [NS, D], bf16, kind="ExternalOutput").ap()

    dram = (xparts, b_fl, exp_m, b_vec, out)
    with tile.TileContext(nc) as tc:
        _kernel_body(nc, tc, dram)
    nc.compile()
    return nc


def host_prep(x, W_gate, A, B, W_base, b_base):
    """Shard + pack the full inputs into 8 per-core input maps.

    All big tensors are packed so each SBUF partition's data is one
    contiguous DRAM run (128 large DMA descriptors per load). x stays
    fp32 (exact gating top-2); wave-0 x rows carry the wg and a chunks
    appended; W_base/B go bf16.
    """
    AUG = 512 + E + ER
    xT = np.ascontiguousarray(x.T)  # [D, N] fp32
    wgC = W_gate.T.reshape(DC, 128, E)  # [c, p, e]
    a_flat = A.transpose(1, 0, 2).reshape(D, ER)
    aC = a_flat.reshape(DC, 128, ER)  # [c, p, r]
    wbT = W_base.T.astype(NP_BF16).reshape(DC, 128, D)  # [c, p, dout]
    b_fl = np.ascontiguousarray(B.reshape(ER, D)).astype(NP_BF16)
    exp_m = np.zeros((E, ER), dtype=np.float32)
    for e in range(E):
        exp_m[e, e * R : (e + 1) * R] = 1.0
    exp_m = exp_m.astype(NP_BF16)
    b_vec = np.ascontiguousarray(b_base.reshape(1, D)).astype(np.float32)

    wbg = {}
    for dt_ in range(DT):
        for par in range(2):
            cs = [par, par + 2, par + 4, par + 6]
            g = np.stack(
                [wbT[c, :, dt_ * 512 : (dt_ + 1) * 512] for c in cs], axis=1
            )
            wbg[(dt_, par)] = np.ascontiguousarray(g)  # [128, 4, 512]

    in_maps = []
    for core in range(CORES):
        xc = xT[:, core * NS : (core + 1) * NS].reshape(DC, 128, NS)
        m = {
            "b_fl": b_fl,
            "exp_m": exp_m,
            "b_vec": b_vec,
            "wb00": wbg[(0, 0)],
            "wb01": wbg[(0, 1)],
            "wb10": wbg[(1, 0)],
            "wb11": wbg[(1, 1)],
        }
        for par in range(2):
            for half in range(2):
                cs = [par + 4 * half, par + 4 * half + 2]
                g0 = np.stack(
                    [
                        np.concatenate(
                            [xc[c, :, 0:512], wgC[c], aC[c]], axis=1
                        )
                        for c in cs
                    ],
                    axis=1,
                )  # [128, 2, AUG]
                m[f"xg0{par}{half}"] = np.ascontiguousarray(
                    g0, dtype=np.float32
                )
                g1 = np.stack([xc[c, :, 512:1024] for c in cs], axis=1)
                m[f"xg1{par}{half}"] = np.ascontiguousarray(
                    g1, dtype=np.float32
                )
        in_maps.append(m)
    return in_maps


def kernel(x, W_gate, A, B, W_base, b_base):
    x = np.asarray(x, dtype=np.float32)
    W_gate = np.asarray(W_gate, dtype=np.float32)
    A = np.asarray(A, dtype=np.float32)
    B = np.asarray(B, dtype=np.float32)
    W_base = np.asarray(W_base, dtype=np.float32)
    b_base = np.asarray(b_base, dtype=np.float32)

    if "nc" not in _CACHE:
        _CACHE["nc"] = build_nc()
    nc = _CACHE["nc"]

    in_maps = host_prep(x, W_gate, A, B, W_base, b_base)
    res = run_bass_kernel_spmd(nc, in_maps, core_ids=list(range(CORES)))
    outs = [
        np.asarray(res.results[c]["out"]).astype(np.float32) for c in range(CORES)
    ]
    return np.concatenate(outs, axis=0)
